# revision 4
# baseline (speedup 1.0000x reference)
"""Two-layer Elman RNN (B=64, S=512, EMB=512, HID=1024) on 8 TRN2 NeuronCores.

Data-parallel over batch: each core processes 8 batch rows end-to-end.
Layout: feature-major activations (h^T: HID on partitions, tokens on free dim),
token columns ordered (t, b) so each scan step reads/writes a contiguous
[128, 8] block. All matmuls in fp16 (fp32 PSUM accumulate), tanh/sigmoid fp32.
"""

import os
from contextlib import ExitStack

import numpy as np

import concourse.bass as bass
import concourse.bacc as bacc
import concourse.mybir as mybir
import concourse.tile as tile
from concourse.bass import IndirectOffsetOnAxis
from concourse.bass_utils import run_bass_kernel_spmd
from concourse.masks import make_identity

P = 128
VOCAB, EMB, HID = 50257, 512, 1024
B, S = 64, 512
NCORES = 8
BL = B // NCORES          # local batch per core
KE = EMB // P             # 4  k-tiles over EMB
KH = HID // P             # 8  k-tiles over HID
M = HID // P              # 8  m-tiles over HID

CDT = mybir.dt.float16
NP_CDT = np.float16
F32 = mybir.dt.float32
I32 = mybir.dt.int32

_TRACE = False            # set by test.py for profiling runs
_BUILT = {}
COLT = int(os.environ.get("KERNEL_COLT", "1"))   # col-tiling factor for scan (1/2/4)


def build(s=S, bl=BL):
    """Build the single-core (replicated SPMD) Bass program."""
    tok = s * bl
    G = tok // P                       # gather groups of 128 tokens
    CC = min(512, tok)                 # gemm chunk columns
    CH = tok // CC
    nc = bacc.Bacc("TRN2", target_bir_lowering=False, debug=False, num_devices=NCORES)

    xg = nc.dram_tensor("xg", [G, P], I32, kind="ExternalInput").ap()
    emb_d = nc.dram_tensor("emb", [VOCAB, EMB], CDT, kind="ExternalInput").ap()
    wi1_d = nc.dram_tensor("wi1", [EMB, HID], CDT, kind="ExternalInput").ap()
    wh1_d = nc.dram_tensor("wh1", [HID, HID], CDT, kind="ExternalInput").ap()
    wi2_d = nc.dram_tensor("wi2", [HID, HID], CDT, kind="ExternalInput").ap()
    wh2_d = nc.dram_tensor("wh2", [HID, HID], CDT, kind="ExternalInput").ap()
    b1_d = nc.dram_tensor("b1m", [M, P], F32, kind="ExternalInput").ap()
    b2_d = nc.dram_tensor("b2m", [M, P], F32, kind="ExternalInput").ap()
    wd_d = nc.dram_tensor("wdk", [P, KH], CDT, kind="ExternalInput").ap()
    bd_d = nc.dram_tensor("bd8", [bl], F32, kind="ExternalInput").ap()
    y_d = nc.dram_tensor("y", [bl], F32, kind="ExternalOutput").ap()

    AF = mybir.ActivationFunctionType

    with tile.TileContext(nc) as tc, ExitStack() as ctx:
        const_pool = ctx.enter_context(tc.tile_pool(name="const", bufs=1))
        wpool = ctx.enter_context(tc.tile_pool(name="weights", bufs=1))
        xet_pool = ctx.enter_context(tc.tile_pool(name="xet", bufs=1))
        arch_pool = ctx.enter_context(tc.tile_pool(name="arch", bufs=1))
        pre_pool = ctx.enter_context(tc.tile_pool(name="pre", bufs=1))

        ident = const_pool.tile([P, P], CDT, name="ident")
        make_identity(nc, ident[:])
        b1t = const_pool.tile([P, M], F32, name="b1t")
        nc.sync.dma_start(out=b1t[:], in_=b1_d.rearrange("m p -> p m"))
        b2t = const_pool.tile([P, M], F32, name="b2t")
        nc.sync.dma_start(out=b2t[:], in_=b2_d.rearrange("m p -> p m"))
        wd_sb = const_pool.tile([P, KH], CDT, name="wd_sb")
        nc.sync.dma_start(out=wd_sb[:], in_=wd_d[:])
        bd_sb = const_pool.tile([P, 1], F32, name="bd_sb")
        nc.sync.dma_start(out=bd_sb[0:bl, 0:1], in_=bd_d[:])

        # weights: block (k*M + m) holds W[k*128:(k+1)*128, m*128:(m+1)*128]
        wi_sb = wpool.tile([P, KH * M * P], CDT, tag="wi", name="wi1_sb")
        for e in range(KE):
            nc.sync.dma_start(out=wi_sb[:, e * HID:(e + 1) * HID],
                              in_=wi1_d[e * P:(e + 1) * P, :])
        wh_sb = wpool.tile([P, KH * M * P], CDT, tag="wh", name="wh1_sb")
        for k in range(KH):
            nc.sync.dma_start(out=wh_sb[:, k * HID:(k + 1) * HID],
                              in_=wh1_d[k * P:(k + 1) * P, :])

        xeT = [xet_pool.tile([P, tok], CDT, name=f"xeT{e}") for e in range(KE)]
        h1T = [arch_pool.tile([P, tok], CDT, name=f"h1T{m}") for m in range(M)]
        pre1 = [pre_pool.tile([P, tok], CDT, tag=f"pre{m}", name=f"pre1_{m}")
                for m in range(M)]

        # ---- Phase A: embedding gather + transpose to feature-major ----
        with tc.tile_pool(name="gather", bufs=4) as gpool, \
             tc.tile_pool(name="tpsum", bufs=4, space="PSUM") as tpsum:
            for g in range(G):
                idx = gpool.tile([P, 1], I32, tag="idx", name=f"idx{g}")
                nc.sync.dma_start(out=idx[:, 0:1], in_=xg[g, :])
                xe_g = gpool.tile([P, EMB], CDT, tag="xe", name=f"xe{g}")
                nc.gpsimd.indirect_dma_start(
                    out=xe_g[:], out_offset=None, in_=emb_d[:],
                    in_offset=IndirectOffsetOnAxis(ap=idx[:, 0:1], axis=0))
                for e in range(KE):
                    pt = tpsum.tile([P, P], CDT, tag="tp", name=f"tp{g}_{e}")
                    nc.tensor.transpose(out=pt[:], in_=xe_g[:, e * P:(e + 1) * P],
                                        identity=ident[:])
                    nc.vector.tensor_copy(out=xeT[e][:, g * P:(g + 1) * P], in_=pt[:])

        # ---- Phase B: pre1 = xe @ Wi1 + b1 ----
        with tc.tile_pool(name="ppB", bufs=3, space="PSUM") as ppool:
            for c in range(CH):
                cs = slice(c * CC, (c + 1) * CC)
                for m in range(M):
                    ps = ppool.tile([P, CC], F32, tag="pp", name=f"ppB{c}_{m}")
                    for e in range(KE):
                        nc.tensor.matmul(
                            ps[:], lhsT=wi_sb[:, (e * M + m) * P:(e * M + m + 1) * P],
                            rhs=xeT[e][:, cs], start=(e == 0), stop=(e == KE - 1))
                    nc.scalar.activation(out=pre1[m][:, cs], in_=ps[:],
                                         func=AF.Identity, bias=b1t[:, m:m + 1])

        # ---- Phase C: scan1 ----
        def scan(s_steps, wsb, pre_t, out_tiles, out_col):
            """out_col(m, t) -> AP column slice for h_t tile m."""
            with tc.tile_pool(name="spsum", bufs=8, space="PSUM") as spsum:
                for t in range(s_steps):
                    for m in range(M):
                        if t == 0:
                            nc.scalar.activation(
                                out=out_col(m, 0), in_=pre_t[m][:, 0:bl], func=AF.Tanh)
                            continue
                        ps = spsum.tile([P, bl], F32, tag="sp", name=f"sp{t}_{m}")
                        cw = P // COLT
                        for k in range(KH):
                            for j in range(COLT):
                                nc.tensor.matmul(
                                    ps[j * cw:(j + 1) * cw, :],
                                    lhsT=wsb[:, (k * M + m) * P + j * cw:
                                             (k * M + m) * P + (j + 1) * cw],
                                    rhs=out_col(k, t - 1),
                                    start=(k == 0), stop=(k == KH - 1),
                                    tile_position=(0, j * cw))
                        nc.vector.tensor_tensor(
                            out=ps[:], in0=ps[:],
                            in1=pre_t[m][:, t * bl:(t + 1) * bl],
                            op=mybir.AluOpType.add)
                        nc.scalar.activation(
                            out=out_col(m, t), in_=ps[:], func=AF.Tanh)

        scan(s, wh_sb, pre1,
             h1T, lambda m, t: h1T[m][:, t * bl:(t + 1) * bl])

        # ---- Phase D: pre2 = h1 @ Wi2 + b2 ----
        wi2_sb = wpool.tile([P, KH * M * P], CDT, tag="wi", name="wi2_sb")
        for k in range(KH):
            nc.sync.dma_start(out=wi2_sb[:, k * HID:(k + 1) * HID],
                              in_=wi2_d[k * P:(k + 1) * P, :])
        pre2 = [pre_pool.tile([P, tok], CDT, tag=f"pre{m}", name=f"pre2_{m}")
                for m in range(M)]
        with tc.tile_pool(name="ppD", bufs=3, space="PSUM") as ppool:
            for c in range(CH):
                cs = slice(c * CC, (c + 1) * CC)
                for m in range(M):
                    ps = ppool.tile([P, CC], F32, tag="pp", name=f"ppD{c}_{m}")
                    for k in range(KH):
                        nc.tensor.matmul(
                            ps[:], lhsT=wi2_sb[:, (k * M + m) * P:(k * M + m + 1) * P],
                            rhs=h1T[k][:, cs], start=(k == 0), stop=(k == KH - 1))
                    nc.scalar.activation(out=pre2[m][:, cs], in_=ps[:],
                                         func=AF.Identity, bias=b2t[:, m:m + 1])

        # ---- Phase E: scan2 (ring buffer, only last h needed) ----
        wh2_sb = wpool.tile([P, KH * M * P], CDT, tag="wh", name="wh2_sb")
        for k in range(KH):
            nc.sync.dma_start(out=wh2_sb[:, k * HID:(k + 1) * HID],
                              in_=wh2_d[k * P:(k + 1) * P, :])
        ring = arch_pool.tile([P, M * 2 * bl], CDT, name="ring")

        def ring_col(m, t):
            q = t % 2
            return ring[:, (m * 2 + q) * bl:(m * 2 + q + 1) * bl]

        scan(s, wh2_sb, pre2, None, ring_col)

        # ---- Phase F: head ----
        with tc.tile_pool(name="hps", bufs=1, space="PSUM") as hpool:
            hps = hpool.tile([bl, 1], F32, name="hps")
            q = (s - 1) % 2
            for k in range(KH):
                nc.tensor.matmul(
                    hps[:], lhsT=ring[:, (k * 2 + q) * bl:(k * 2 + q + 1) * bl],
                    rhs=wd_sb[:, k:k + 1], start=(k == 0), stop=(k == KH - 1))
            y_sb = const_pool.tile([P, 1], F32, name="y_sb")
            nc.scalar.activation(out=y_sb[0:bl, 0:1], in_=hps[:],
                                 func=AF.Sigmoid, bias=bd_sb[0:bl, 0:1])
            nc.sync.dma_start(out=y_d[:], in_=y_sb[0:bl, 0:1])

    nc.compile()
    return nc


def _prep_maps(x, emb, Wi1, Wh1, b1, Wi2, Wh2, b2, Wd, bd, s=S, bl=BL):
    bf = NP_CDT
    x = np.asarray(x, np.int32)
    shared = {
        "emb": np.ascontiguousarray(np.asarray(emb, bf)),
        "wi1": np.ascontiguousarray(np.asarray(Wi1, bf)),
        "wh1": np.ascontiguousarray(np.asarray(Wh1, bf)),
        "wi2": np.ascontiguousarray(np.asarray(Wi2, bf)),
        "wh2": np.ascontiguousarray(np.asarray(Wh2, bf)),
        "b1m": np.ascontiguousarray(np.asarray(b1, np.float32).reshape(M, P)),
        "b2m": np.ascontiguousarray(np.asarray(b2, np.float32).reshape(M, P)),
        "wdk": np.ascontiguousarray(np.asarray(Wd, bf).reshape(KH, P).T),
        "bd8": np.ascontiguousarray(np.broadcast_to(np.asarray(bd, np.float32), (bl,))),
    }
    in_maps = []
    for c in range(NCORES):
        xs = x[c * bl:(c + 1) * bl, :s]           # [bl, s]
        xgrp = np.ascontiguousarray(xs.T).reshape(-1, P)  # (t, b) order, groups of 128
        in_maps.append({**shared, "xg": xgrp})
    return in_maps


def kernel(x, emb, Wi1, Wh1, b1, Wi2, Wh2, b2, Wd, bd):
    key = (S, BL)
    if key not in _BUILT:
        _BUILT[key] = build()
    nc = _BUILT[key]
    in_maps = _prep_maps(x, emb, Wi1, Wh1, b1, Wi2, Wh2, b2, Wd, bd)
    res = run_bass_kernel_spmd(nc, in_maps, list(range(NCORES)), trace=_TRACE)
    kernel.last_result = res
    y = np.concatenate([np.asarray(res.results[c]["y"], np.float32)
                        for c in range(NCORES)])
    return y


# revision 10
# speedup vs baseline: 1.6417x; 1.6417x over previous
"""Two-layer Elman RNN (B=64, S=512, EMB=512, HID=1024) on 8 TRN2 NeuronCores.

Layer-pipelined pairs: pair p = (core p, core p+4) handles batch quarter p
(16 rows). Core p runs the layer-1 scan and streams pre2 = h1 @ Wi2 + b2
chunks (32 steps each) to core p+4 via pair-wise AllGather; core p+4 runs the
layer-2 scan one chunk behind and produces the output quarter. All 8 cores
execute an identical SPMD program — roles differ only in input data (scan
weights Wh1 vs Wh2, and blend scalars alpha/beta selecting local-pre1 vs
received-pre2 as the scan input).

Layout: feature-major activations (h^T: HID on partitions, tokens on free
dim), token columns ordered (t, b) so a scan step reads/writes a contiguous
[128, 16] block. Matmuls in fp16 (fp32 PSUM accumulate), tanh/sigmoid fp32.
The zero fixed point of h = tanh(W h + 0) makes the warm-up chunk of the
layer-2 cores (fed zeros) end exactly in the correct initial state h = 0.
"""

import os
from contextlib import ExitStack

import numpy as np

import concourse.bass as bass
import concourse.bacc as bacc
import concourse.mybir as mybir
import concourse.tile as tile
from concourse.bass import IndirectOffsetOnAxis
from concourse.bass_utils import run_bass_kernel_spmd
from concourse.masks import make_identity

P = 128
VOCAB, EMB, HID = 50257, 512, 1024
B, S = 64, 512
NCORES = 8
NPAIR = 4
BL = B // NPAIR           # batch rows per pair = 16
KE = EMB // P             # 4
KH = HID // P             # 8
M = HID // P              # 8
CS = 32                   # scan steps per chunk
NCH = S // CS             # 16 chunks
CCOL = BL * CS            # 512 columns per chunk

CDT = mybir.dt.float16
NP_CDT = np.float16
F32 = mybir.dt.float32
I32 = mybir.dt.int32

_BUILT = {}
REPLICA_GROUPS = [[p, p + NPAIR] for p in range(NPAIR)]


def build(local_cc=False):
    """local_cc=True replaces the AllGather with an equivalent-volume local
    DMA so the collective-free program can run under TimelineSim."""
    tok = S * BL                      # 8192 token-columns per pair
    nc = bacc.Bacc("TRN2", target_bir_lowering=False, debug=False, num_devices=NCORES)

    xg = nc.dram_tensor("xg", [tok // P, P], I32, kind="ExternalInput").ap()
    emb_d = nc.dram_tensor("emb", [VOCAB, EMB], CDT, kind="ExternalInput").ap()
    wi1_d = nc.dram_tensor("wi1", [EMB, HID], CDT, kind="ExternalInput").ap()
    whs_d = nc.dram_tensor("whs", [HID, HID], CDT, kind="ExternalInput").ap()
    wsend_d = nc.dram_tensor("wsend", [HID, HID], CDT, kind="ExternalInput").ap()
    b1_d = nc.dram_tensor("b1m", [M, P], F32, kind="ExternalInput").ap()
    bs_d = nc.dram_tensor("bsm", [M, P], F32, kind="ExternalInput").ap()
    ab_d = nc.dram_tensor("ab", [2, P], F32, kind="ExternalInput").ap()
    wd_d = nc.dram_tensor("wdk", [P, KH], CDT, kind="ExternalInput").ap()
    bd_d = nc.dram_tensor("bdv", [BL], F32, kind="ExternalInput").ap()
    y_d = nc.dram_tensor("y", [BL], F32, kind="ExternalOutput").ap()

    AF = mybir.ActivationFunctionType
    ALU = mybir.AluOpType

    with tile.TileContext(nc) as tc, ExitStack() as ctx:
        const_pool = ctx.enter_context(tc.tile_pool(name="const", bufs=1))
        wpool = ctx.enter_context(tc.tile_pool(name="weights", bufs=1))
        dpool = ctx.enter_context(tc.tile_pool(name="dram", bufs=1, space="DRAM"))
        cpool = ctx.enter_context(tc.tile_pool(name="ccdram", bufs=2, space="DRAM"))
        gpool = ctx.enter_context(tc.tile_pool(name="gather", bufs=4))
        xpool = ctx.enter_context(tc.tile_pool(name="xet", bufs=2))
        lpool = ctx.enter_context(tc.tile_pool(name="locpre", bufs=2))
        prepool = ctx.enter_context(tc.tile_pool(name="prework", bufs=2))
        rpool = ctx.enter_context(tc.tile_pool(name="recv", bufs=2))
        apool = ctx.enter_context(tc.tile_pool(name="arch", bufs=2))
        spool = ctx.enter_context(tc.tile_pool(name="send", bufs=2))
        bigps = ctx.enter_context(tc.tile_pool(name="bigps", bufs=3, space="PSUM"))
        spsum = ctx.enter_context(tc.tile_pool(name="spsum", bufs=4, space="PSUM"))

        ident = const_pool.tile([P, P], CDT, name="ident")
        make_identity(nc, ident[:])
        b1t = const_pool.tile([P, M], F32, name="b1t")
        nc.sync.dma_start(out=b1t[:], in_=b1_d.rearrange("m p -> p m"))
        bst = const_pool.tile([P, M], F32, name="bst")
        nc.sync.dma_start(out=bst[:], in_=bs_d.rearrange("m p -> p m"))
        abt = const_pool.tile([P, 2], F32, name="abt")
        nc.sync.dma_start(out=abt[:], in_=ab_d.rearrange("a p -> p a"))
        alpha, beta = abt[:, 0:1], abt[:, 1:2]
        wd_sb = const_pool.tile([P, KH], CDT, name="wd_sb")
        nc.sync.dma_start(out=wd_sb[:], in_=wd_d[:])
        bd_sb = const_pool.tile([P, 1], F32, name="bd_sb")
        nc.sync.dma_start(out=bd_sb[0:BL, 0:1], in_=bd_d[:])

        wi_sb = wpool.tile([P, KE * M * P], CDT, name="wi_sb")
        for e in range(KE):
            nc.sync.dma_start(out=wi_sb[:, e * HID:(e + 1) * HID],
                              in_=wi1_d[e * P:(e + 1) * P, :])
        whs_sb = wpool.tile([P, KH * M * P], CDT, name="whs_sb")
        for k in range(KH):
            nc.sync.dma_start(out=whs_sb[:, k * HID:(k + 1) * HID],
                              in_=whs_d[k * P:(k + 1) * P, :])
        wsend_sb = wpool.tile([P, KH * M * P], CDT, name="wsend_sb")
        for k in range(KH):
            nc.sync.dma_start(out=wsend_sb[:, k * HID:(k + 1) * HID],
                              in_=wsend_d[k * P:(k + 1) * P, :])

        # local pre1 staging in DRAM, per m-tile
        pre1_dram = [dpool.tile([P, tok], CDT, space="DRAM", name=f"pre1d{m}")
                     for m in range(M)]

        zrecv = [const_pool.tile([P, CCOL], CDT, name=f"zrecv{m}") for m in range(M)]
        for m in range(M):
            nc.vector.memset(zrecv[m][:], 0.0)

        # ---- Phase A+B: embed + pre1 per chunk, staged to DRAM ----
        for c in range(NCH):
            xeT = [xpool.tile([P, CCOL], CDT, tag=f"xeT{e}", name=f"xeT{c}_{e}")
                   for e in range(KE)]
            for gi in range(CCOL // P):
                g = c * (CCOL // P) + gi
                idx = gpool.tile([P, 1], I32, tag="idx", name=f"idx{g}")
                nc.sync.dma_start(out=idx[:, 0:1], in_=xg[g, :])
                xe_g = gpool.tile([P, EMB], CDT, tag="xe", name=f"xe{g}")
                nc.gpsimd.indirect_dma_start(
                    out=xe_g[:], out_offset=None, in_=emb_d[:],
                    in_offset=IndirectOffsetOnAxis(ap=idx[:, 0:1], axis=0))
                for e in range(KE):
                    pt = bigps.tile([P, P], CDT, tag="ps", name=f"tp{g}_{e}")
                    nc.tensor.transpose(out=pt[:], in_=xe_g[:, e * P:(e + 1) * P],
                                        identity=ident[:])
                    nc.vector.tensor_copy(out=xeT[e][:, gi * P:(gi + 1) * P],
                                          in_=pt[:])
            for m in range(M):
                ps = bigps.tile([P, CCOL], F32, tag="ps", name=f"ppB{c}_{m}")
                for e in range(KE):
                    nc.tensor.matmul(
                        ps[:], lhsT=wi_sb[:, (e * M + m) * P:(e * M + m + 1) * P],
                        rhs=xeT[e][:, :], start=(e == 0), stop=(e == KE - 1))
                pc = lpool.tile([P, CCOL], CDT, tag=f"pb{m}", name=f"preb{c}_{m}")
                nc.scalar.activation(out=pc[:], in_=ps[:], func=AF.Identity,
                                     bias=b1t[:, m:m + 1])
                nc.sync.dma_start(out=pre1_dram[m][:, c * CCOL:(c + 1) * CCOL],
                                  in_=pc[:])

        # ---- Main pipelined loop ----
        arch_prev = None
        recv_prev = zrecv
        for c in range(NCH + 1):
            lc = min(c, NCH - 1)
            # stream in local pre1 chunk, blend with received chunk
            PRE = []
            for m in range(M):
                loc = lpool.tile([P, CCOL], CDT, tag=f"loc{m}", name=f"loc{c}_{m}")
                nc.sync.dma_start(out=loc[:],
                                  in_=pre1_dram[m][:, lc * CCOL:(lc + 1) * CCOL])
                tmp = prepool.tile([P, CCOL], CDT, tag=f"tmp{m}", name=f"tmp{c}_{m}")
                nc.vector.tensor_scalar_mul(tmp[:], recv_prev[m][:], beta)
                pre = prepool.tile([P, CCOL], CDT, tag=f"PRE{m}", name=f"PRE{c}_{m}")
                nc.vector.scalar_tensor_tensor(
                    out=pre[:], in0=loc[:], scalar=alpha, in1=tmp[:],
                    op0=ALU.mult, op1=ALU.add)
                PRE.append(pre)

            # scan CS steps
            arch = [apool.tile([P, CCOL], CDT, tag=f"arch{m}", name=f"arch{c}_{m}")
                    for m in range(M)]
            for t in range(CS):
                for m in range(M):
                    if c == 0 and t == 0:
                        nc.scalar.activation(out=arch[m][:, 0:BL],
                                             in_=PRE[m][:, 0:BL], func=AF.Tanh)
                        continue
                    ps = spsum.tile([P, BL], F32, tag="sp", name=f"sp{c}_{t}_{m}")
                    for k in range(KH):
                        rk = (arch[k][:, (t - 1) * BL:t * BL] if t > 0
                              else arch_prev[k][:, (CS - 1) * BL:CS * BL])
                        nc.tensor.matmul(
                            ps[:], lhsT=whs_sb[:, (k * M + m) * P:(k * M + m + 1) * P],
                            rhs=rk, start=(k == 0), stop=(k == KH - 1))
                    nc.vector.tensor_tensor(
                        out=ps[:], in0=ps[:], in1=PRE[m][:, t * BL:(t + 1) * BL],
                        op=ALU.add)
                    nc.scalar.activation(out=arch[m][:, t * BL:(t + 1) * BL],
                                         in_=ps[:], func=AF.Tanh)
            arch_prev = arch

            if c == NCH:
                break

            # chunk matmul: send = arch @ Wsend + bsend, then pair AllGather
            send_db = cpool.tile([HID, CCOL], CDT, space="DRAM",
                                 name=f"send_db{c}")
            for m in range(M):
                ps = bigps.tile([P, CCOL], F32, tag="ps", name=f"ppS{c}_{m}")
                for k in range(KH):
                    nc.tensor.matmul(
                        ps[:], lhsT=wsend_sb[:, (k * M + m) * P:(k * M + m + 1) * P],
                        rhs=arch[k][:, :], start=(k == 0), stop=(k == KH - 1))
                snd = spool.tile([P, CCOL], CDT, tag=f"snd{m}", name=f"snd{c}_{m}")
                nc.scalar.activation(out=snd[:], in_=ps[:], func=AF.Identity,
                                     bias=bst[:, m:m + 1])
                nc.sync.dma_start(out=send_db[m * P:(m + 1) * P, :], in_=snd[:])

            recv_db = cpool.tile([2 * HID, CCOL], CDT, space="DRAM",
                                 name=f"recv_db{c}")
            if local_cc:
                nc.gpsimd.dma_start(out=recv_db[0:HID, :], in_=send_db[:])
            else:
                nc.gpsimd.collective_compute(
                    "AllGather", ALU.bypass, ins=[send_db[:]], outs=[recv_db[:]],
                    replica_groups=REPLICA_GROUPS)
            recv = []
            for m in range(M):
                rv = rpool.tile([P, CCOL], CDT, tag=f"rv{m}", name=f"rv{c}_{m}")
                nc.sync.dma_start(out=rv[:], in_=recv_db[m * P:(m + 1) * P, :])
                recv.append(rv)
            recv_prev = recv

        # ---- head ----
        with tc.tile_pool(name="hps", bufs=1, space="PSUM") as hpool:
            hps = hpool.tile([BL, 1], F32, name="hps")
            for k in range(KH):
                nc.tensor.matmul(
                    hps[:], lhsT=arch_prev[k][:, (CS - 1) * BL:CS * BL],
                    rhs=wd_sb[:, k:k + 1], start=(k == 0), stop=(k == KH - 1))
            y_sb = const_pool.tile([P, 1], F32, name="y_sb")
            nc.scalar.activation(out=y_sb[0:BL, 0:1], in_=hps[:],
                                 func=AF.Sigmoid, bias=bd_sb[0:BL, 0:1])
            nc.sync.dma_start(out=y_d[:], in_=y_sb[0:BL, 0:1])

    nc.compile()
    return nc


def _prep_maps(x, emb, Wi1, Wh1, b1, Wi2, Wh2, b2, Wd, bd):
    f = NP_CDT
    x = np.asarray(x, np.int32)
    shared = {
        "emb": np.ascontiguousarray(np.asarray(emb, f)),
        "wi1": np.ascontiguousarray(np.asarray(Wi1, f)),
        "wsend": np.ascontiguousarray(np.asarray(Wi2, f)),
        "b1m": np.ascontiguousarray(np.asarray(b1, np.float32).reshape(M, P)),
        "bsm": np.ascontiguousarray(np.asarray(b2, np.float32).reshape(M, P)),
        "wdk": np.ascontiguousarray(np.asarray(Wd, f).reshape(KH, P).T),
        "bdv": np.ascontiguousarray(np.broadcast_to(
            np.asarray(bd, np.float32), (BL,))),
    }
    wh1 = np.ascontiguousarray(np.asarray(Wh1, f))
    wh2 = np.ascontiguousarray(np.asarray(Wh2, f))
    ab_a = np.stack([np.ones(P, np.float32), np.zeros(P, np.float32)])
    ab_b = np.stack([np.zeros(P, np.float32), np.ones(P, np.float32)])
    in_maps = []
    for c in range(NCORES):
        p = c % NPAIR
        xs = x[p * BL:(p + 1) * BL, :]                    # [16, 512]
        xgrp = np.ascontiguousarray(xs.T).reshape(-1, P)  # (t, b) order
        role_a = c < NPAIR
        in_maps.append({
            **shared,
            "xg": xgrp,
            "whs": wh1 if role_a else wh2,
            "ab": ab_a if role_a else ab_b,
        })
    return in_maps


def kernel(x, emb, Wi1, Wh1, b1, Wi2, Wh2, b2, Wd, bd):
    if "nc" not in _BUILT:
        _BUILT["nc"] = build()
    nc = _BUILT["nc"]
    in_maps = _prep_maps(x, emb, Wi1, Wh1, b1, Wi2, Wh2, b2, Wd, bd)
    res = run_bass_kernel_spmd(nc, in_maps, list(range(NCORES)))
    kernel.last_result = res
    y = np.concatenate([np.asarray(res.results[NPAIR + p]["y"], np.float32)
                        for p in range(NPAIR)])
    return y
